# revision 3
# baseline (speedup 1.0000x reference)
"""DLRM DotInteraction kernel for Trainium2 (Bass/Tile), 8-core data parallel.

Problem: dense_feature [B=16384, D=128] f32, sparse_stack [S=26, B, D] f32.
cat = [dense; sparse] per sample -> [B, N=27, D]; G_b = cat_b @ cat_b^T;
out = [dense | tril(G_b) (378 vals, row-major incl diag)] -> [B, 506] f32.

Per core (B_c = 2048 samples), pipelined over supertiles of 4x128-sample tiles:
  1. dense loaded once as f32 [128, 16, 128] (stored straight back out and
     cast once per tile to fp16); sparse loaded per tile via SWDGE cast-DMA
     (f32 HBM -> fp16 SBUF, natural [sample, feature, d] layout).
  2. TensorE transpose of each feature slab [128 s, 128 d] -> [128 d, 128 s]
     (fp16 vs identity), packed 4 slabs per PSUM bank, copied to SBUF
     xt[d, j, s] by DVE/ScalarE.
  3. TensorE Gram, 4 samples per matmul: stationary = 128 cols packing
     4 samples x 32 features (27 + 5 garbage pad -> FWL-eligible), moving =
     4 samples x 27 features (108 cols). Only the block-diagonal 32x27
     output blocks are kept; cross-sample blocks are dead compute that
     costs no extra PE cycles. 512 matmuls/core vs 2048 per-sample ones.
  4. DVE/ScalarE copy diagonal blocks PSUM -> SBUF gcol[i, c, g, tp, j].
  5. Flatten: 27 DMAs per supertile gather tril rows into rowq
     [sample partition, 378]; one 774 KB tri store + one 256 KB dense
     store per supertile.
"""

import numpy as np

import concourse.bacc as bacc
import concourse.mybir as mybir
import concourse.tile as tile
from concourse import bass_utils
from concourse.masks import make_identity

B = 16384
D = 128
S = 26
N = S + 1  # 27
NCORES = 8
BC = B // NCORES  # 2048 samples per core
PT = 128  # samples per sbuf tile
TRI = N * (N + 1) // 2  # 378
W = D + TRI  # 506
TPS = 4  # tiles per supertile
GR = 8  # sample-groups (of 4 samples) per psum round

f32 = mybir.dt.float32
f16 = mybir.dt.float16


def build_kernel(b_core: int = BC, reps: int = 1):
    nc = bacc.Bacc("TRN2", target_bir_lowering=False, debug=False)
    dense = nc.dram_tensor("dense", [b_core, D], f32, kind="ExternalInput").ap()
    sparse = nc.dram_tensor("sparse", [S, b_core, D], f32, kind="ExternalInput").ap()
    out = nc.dram_tensor("out", [b_core, W], f32, kind="ExternalOutput").ap()

    t_total = b_core // PT  # 16
    tps = min(TPS, t_total)
    n_super = t_total // tps
    gpt = PT // 4  # 32 groups of 4 samples per tile
    rpt = gpt // GR  # psum rounds per tile

    with tile.TileContext(nc) as tc:
        with (
            tc.tile_pool(name="singles", bufs=1) as singles,
            tc.tile_pool(name="dense32", bufs=2) as dense_pool,
            tc.tile_pool(name="nat", bufs=3) as nat_pool,
            tc.tile_pool(name="xt", bufs=3) as xt_pool,
            tc.tile_pool(name="gcol", bufs=1) as gcol_pool,
            tc.tile_pool(name="row", bufs=2) as row_pool,
            tc.tile_pool(name="psum", bufs=2, space="PSUM") as psum_pool,
            tc.tile_pool(name="psumt", bufs=3, space="PSUM") as psumt_pool,
        ):
            id16 = singles.tile([128, 128], f16, name="id16")
            make_identity(nc, id16)

            for _rep in range(reps):
                # dense, f32, sample (16t + p): feeds both the passthrough
                # store and (via one cast per tile) Gram feature 0.
                db = dense_pool.tile([128, t_total, D], f32)
                nc.sync.dma_start(
                    out=db,
                    in_=dense.rearrange("(t p) d -> p t d", p=PT),
                )

                for st in range(n_super):
                    # gcol[i, c, g, tp, j] = G[sample 32c+g of tile tp][i, j]
                    gcol = gcol_pool.tile([32, 4, gpt, tps, N], f32)
                    # rowq[p, tp, :] = tril row of sample (st, tp, p)
                    rowq = row_pool.tile([128, tps, TRI], f32)

                    for tp in range(tps):
                        t = st * tps + tp
                        rows = slice(t * PT, (t + 1) * PT)
                        # --- load sparse (cast f32->f16 in DMA), dense cast ---
                        nat = nat_pool.tile([128, N, D], f16)
                        nc.gpsimd.dma_start(
                            out=nat[:, 1:N, :],
                            in_=sparse[:, rows, :].rearrange("s b d -> b s d"),
                        )
                        nc.scalar.copy(out=nat[:, 0, :], in_=db[:, t, :])

                        # --- TensorE transpose of each feature slab ---
                        # xt[d, g, 32c+i] = feature i of sample 32c+g: each
                        # group's 128 stationary cols are contiguous (single
                        # free dim, FWL-eligible full-width weight load).
                        xt = xt_pool.tile([128, gpt, PT], f16)
                        # scatter view [d, j(4), c, g] for the PSUM copy-back
                        xs = xt.rearrange("d g (c i) -> d i c g", c=4)
                        for k in range(7):  # 4-slab packs: 6*4 + 3
                            j0 = 4 * k
                            nj = min(4, N - j0)
                            pt_ = psumt_pool.tile([128, 4, PT], f16, tag="pt")
                            for jj in range(nj):
                                nc.tensor.transpose(
                                    pt_[:, jj, :], nat[:, j0 + jj, :], id16
                                )
                            cp = nc.vector.tensor_copy if k % 2 else nc.scalar.copy
                            cp(
                                out=xs[:, j0 : j0 + nj, :, :],
                                in_=pt_[:, 0:nj, :],
                            )

                        # --- Gram matmuls: 4 samples per matmul ---
                        # stationary = moving = xt[:, g, :]: psum partition
                        # 32c+i x col 32c'+j holds <x_i of sample 32c+g,
                        # x_j of sample 32c'+g>; only the diagonal c==c'
                        # 32x27 blocks are kept (cols/partitions with i>=27
                        # are garbage pad, also dropped).
                        for r in range(rpt):
                            ps = psum_pool.tile([128, GR, 128], f32)
                            for q in range(GR):
                                g = r * GR + q
                                nc.tensor.matmul(
                                    out=ps[:, q, :],
                                    lhsT=xt[:, g, :],
                                    rhs=xt[:, g, :],
                                    start=True,
                                    stop=True,
                                )
                            off = r * GR
                            for c in range(4):
                                cp2 = (
                                    nc.vector.tensor_copy if c % 2 else nc.scalar.copy
                                )
                                cp2(
                                    out=gcol[:, c, off : off + GR, tp, :],
                                    in_=ps[
                                        32 * c : 32 * c + 32, :, 32 * c : 32 * c + N
                                    ],
                                )

                    # --- flatten: Gram row i of sample (tp, 32c+g) from
                    # gcol[i, c, g, tp, 0:i+1] to rowq[32c+g, tp, toff:] ---
                    for i in range(N):
                        toff = i * (i + 1) // 2
                        eng = nc.scalar if i % 2 == 0 else nc.sync
                        eng.dma_start(
                            out=rowq[:, :, toff : toff + i + 1],
                            in_=gcol[i : i + 1, :, :, :, 0 : i + 1],
                        )

                    # --- stores: tri (774 KB) + dense passthrough (256 KB) ---
                    rows4 = slice(st * tps * PT, (st + 1) * tps * PT)
                    nc.scalar.dma_start(
                        out=out[rows4, D:W].rearrange("(tp p) w -> p tp w", p=PT),
                        in_=rowq,
                    )
                    nc.sync.dma_start(
                        out=out[rows4, 0:D].rearrange("(tp p) d -> p tp d", p=PT),
                        in_=db[:, st * tps : (st + 1) * tps, :],
                    )

    nc.compile()
    return nc


_CACHE: dict = {}


def _get_nc():
    if "nc" not in _CACHE:
        _CACHE["nc"] = build_kernel(BC)
    return _CACHE["nc"]


def kernel(dense_feature, sparse_stack, **run_kwargs):
    dense_feature = np.asarray(dense_feature, dtype=np.float32)
    sparse_stack = np.asarray(sparse_stack, dtype=np.float32)
    assert dense_feature.shape == (B, D)
    assert sparse_stack.shape == (S, B, D)

    nc = run_kwargs.pop("nc", None) or _get_nc()
    in_maps = []
    for ci in range(NCORES):
        sl = slice(ci * BC, (ci + 1) * BC)
        in_maps.append(
            {
                "dense": np.ascontiguousarray(dense_feature[sl]),
                "sparse": np.ascontiguousarray(sparse_stack[:, sl, :]),
            }
        )
    res = bass_utils.run_bass_kernel_spmd(
        nc, in_maps, core_ids=list(range(NCORES)), **run_kwargs
    )
    out = np.concatenate([r["out"] for r in res.results], axis=0)
    if run_kwargs:
        _CACHE["last_result"] = res
    return out


# revision 4
# speedup vs baseline: 2.2498x; 2.2498x over previous
"""DLRM DotInteraction kernel for Trainium2 (Bass/Tile), 8-core data parallel.

Problem: dense_feature [B=16384, D=128] f32, sparse_stack [S=26, B, D] f32.
cat = [dense; sparse] per sample -> [B, N=27, D]; G_b = cat_b @ cat_b^T;
out = [dense | tril(G_b) (378 vals, row-major incl diag)] -> [B, 506] f32.

Per core (B_c = 2048 samples), pipelined over supertiles of 4x128-sample
tiles:
  1. dense loaded once f32 -> db [128, 16, 128] (stored straight back out,
     and cast per tile to fp16 for Gram feature 0). Sparse per tile: even
     tiles SWDGE cast-DMA (f32 HBM -> fp16 SBUF), odd tiles HWDGE f32 +
     ScalarE cast (splits work across DMA paths and engines).
  2. TensorE transpose per feature slab [128 s, 128 d] -> [128 d, 128 s]
     fp16 vs identity, packed 8 slabs per PSUM bank, copied to SBUF
     xt[d, j, s] by DVE/ScalarE (contiguous, 2x fp16 mode).
  3. TensorE Gram per sample: group g = samples {32c + g}; 4 col-tiled
     matmuls (tile_position (0,32c)), K=128 d, M=32 (27+5 pad), N=27.
  4. DVE/ScalarE copy Gram PSUM -> SBUF gcol fp16 [i, c, g, tp, j].
  5. Flatten: 27 DMAs per supertile gather tril rows (fp16) into rowq;
     tri store via SWDGE cast-DMA fp16 -> f32 HBM; dense store from db.
"""

import numpy as np

import concourse.bacc as bacc
import concourse.mybir as mybir
import concourse.tile as tile
from concourse import bass_utils
from concourse.masks import make_identity

B = 16384
D = 128
S = 26
N = S + 1  # 27
NCORES = 8
BC = B // NCORES  # 2048 samples per core
PT = 128
TRI = N * (N + 1) // 2  # 378
W = D + TRI  # 506
TPS = 4  # tiles per supertile

f32 = mybir.dt.float32
f16 = mybir.dt.float16

# tunables
GCOL_F16 = True
GCOL_BUFS = 2


def build_kernel(b_core: int = BC, reps: int = 1):
    nc = bacc.Bacc("TRN2", target_bir_lowering=False, debug=False)
    dense = nc.dram_tensor("dense", [b_core, D], f32, kind="ExternalInput").ap()
    sparse = nc.dram_tensor("sparse", [S, b_core, D], f32, kind="ExternalInput").ap()
    out = nc.dram_tensor("out", [b_core, W], f32, kind="ExternalOutput").ap()

    t_total = b_core // PT  # 16
    tps = min(TPS, t_total)
    n_super = t_total // tps
    gpt = PT // 4  # 32

    gdt = f16 if GCOL_F16 else f32

    with tile.TileContext(nc) as tc:
        with (
            tc.tile_pool(name="singles", bufs=1) as singles,
            tc.tile_pool(name="dense32", bufs=2) as dense_pool,
            tc.tile_pool(name="nat32", bufs=2) as nat32_pool,
            tc.tile_pool(name="nat", bufs=3) as nat_pool,
            tc.tile_pool(name="xt", bufs=3) as xt_pool,
            tc.tile_pool(name="gcol", bufs=GCOL_BUFS) as gcol_pool,
            tc.tile_pool(name="row", bufs=2) as row_pool,
            tc.tile_pool(name="psum", bufs=3, space="PSUM") as psum_pool,
            tc.tile_pool(name="psumt", bufs=3, space="PSUM") as psumt_pool,
        ):
            id16 = singles.tile([128, 128], f16, name="id16")
            make_identity(nc, id16)

            for _rep in range(reps):
                db = dense_pool.tile([128, t_total, D], f32)
                nc.sync.dma_start(
                    out=db, in_=dense.rearrange("(t p) d -> p t d", p=PT)
                )
                for st in range(n_super):
                    # gcol[i, c, g, tp, j] = G[sample 32c+g of tile tp][i, j]
                    gcol = gcol_pool.tile([32, 4, gpt, tps, N], gdt)
                    rowq = row_pool.tile([128, tps, TRI], gdt)

                    for tp in range(tps):
                        t = st * tps + tp
                        rows = slice(t * PT, (t + 1) * PT)
                        nat = nat_pool.tile([128, N, D], f16)
                        if t % 2 == 1:
                            nat32 = nat32_pool.tile([128, S, D], f32)
                            nc.sync.dma_start(
                                out=nat32,
                                in_=sparse[:, rows, :].rearrange("s b d -> b s d"),
                            )
                            nc.scalar.copy(out=nat[:, 1:N, :], in_=nat32)
                        else:
                            nc.gpsimd.dma_start(
                                out=nat[:, 1:N, :],
                                in_=sparse[:, rows, :].rearrange("s b d -> b s d"),
                            )
                        nc.scalar.copy(out=nat[:, 0, :], in_=db[:, t, :])

                        # --- TensorE transpose, 8-slab PSUM packs ---
                        xt = xt_pool.tile([128, 32, PT], f16)
                        for k in range(4):  # 3*8 + 3
                            j0 = 8 * k
                            nj = min(8, N - j0)
                            pt_ = psumt_pool.tile([128, 8, PT], f16, tag="pt")
                            for jj in range(nj):
                                nc.tensor.transpose(
                                    pt_[:, jj, :], nat[:, j0 + jj, :], id16
                                )
                            cp = nc.vector.tensor_copy if k % 2 else nc.scalar.copy
                            cp(out=xt[:, j0 : j0 + nj, :], in_=pt_[:, 0:nj, :])

                        # --- per-sample Gram, 4 col-tiled matmuls ---
                        for r in range(2):
                            ps = psum_pool.tile([128, 16, N], f32)
                            for q in range(16):
                                g = r * 16 + q
                                for c in range(4):
                                    sl = 32 * c + g
                                    nc.tensor.matmul(
                                        out=ps[32 * c : 32 * c + 32, q, :],
                                        lhsT=xt[:, :, sl],
                                        rhs=xt[:, 0:N, sl],
                                        start=True,
                                        stop=True,
                                        tile_position=(0, 32 * c),
                                    )
                            off = r * 16
                            for c in range(4):
                                cp2 = (
                                    nc.vector.tensor_copy if c % 2 else nc.scalar.copy
                                )
                                cp2(
                                    out=gcol[:, c, off : off + 16, tp, :],
                                    in_=ps[32 * c : 32 * c + 32, :, :],
                                )

                    # --- flatten tril rows into rowq ---
                    for i in range(N):
                        toff = i * (i + 1) // 2
                        eng = nc.scalar if i % 3 == 0 else nc.sync
                        eng.dma_start(
                            out=rowq[:, :, toff : toff + i + 1],
                            in_=gcol[i : i + 1, :, :, :, 0 : i + 1],
                        )

                    # --- stores ---
                    rows4 = slice(st * tps * PT, (st + 1) * tps * PT)
                    tri_out = out[rows4, D:W].rearrange("(tp p) w -> p tp w", p=PT)
                    if GCOL_F16:
                        nc.gpsimd.dma_start(out=tri_out, in_=rowq)  # cast store
                    else:
                        nc.scalar.dma_start(out=tri_out, in_=rowq)
                    nc.sync.dma_start(
                        out=out[rows4, 0:D].rearrange("(tp p) d -> p tp d", p=PT),
                        in_=db[:, st * tps : (st + 1) * tps, :],
                    )

    nc.compile()
    return nc


_CACHE: dict = {}


def _get_nc():
    if "nc" not in _CACHE:
        _CACHE["nc"] = build_kernel(BC)
    return _CACHE["nc"]


def kernel(dense_feature, sparse_stack, **run_kwargs):
    dense_feature = np.asarray(dense_feature, dtype=np.float32)
    sparse_stack = np.asarray(sparse_stack, dtype=np.float32)
    assert dense_feature.shape == (B, D)
    assert sparse_stack.shape == (S, B, D)

    nc = run_kwargs.pop("nc", None) or _get_nc()
    in_maps = []
    for ci in range(NCORES):
        sl = slice(ci * BC, (ci + 1) * BC)
        in_maps.append(
            {
                "dense": np.ascontiguousarray(dense_feature[sl]),
                "sparse": np.ascontiguousarray(sparse_stack[:, sl, :]),
            }
        )
    res = bass_utils.run_bass_kernel_spmd(
        nc, in_maps, core_ids=list(range(NCORES)), **run_kwargs
    )
    out = np.concatenate([r["out"] for r in res.results], axis=0)
    if run_kwargs:
        _CACHE["last_result"] = res
    return out
